# revision 24
# baseline (speedup 1.0000x reference)
"""AllostericGNN Trainium2 kernel (8 NeuronCores, SPMD) — slot-major attention.

Strategy: shard nodes (and their in-edges, grouped by dst) across 8 cores.
Nodes are degree-sorted and striped across cores so all cores see an identical
degree profile (the compiled program is shared); the inverse permutation is
applied on the host at output assembly. Per 128-dst tile, in-edges live in
"slots" along the free axis of the dst partition: the self edge is slot 0 and
edge slots follow, so ONE SWDGE dma_gather per tile-pass covers self+edges
(s-major 16-wrapped int16 indices, single_packet=False, 4 SWDGE queues
round-robin; Q7 descriptor emission is ~5.3ns/idx flat — merging gathers
bought nothing and fp8 wire + upcast-DMA stole SDMA bandwidth from gathers:
both measured dead ends). Pad slots point at valid self rows and are masked
after exp. Scores are a broadcast multiply + log-tree of packed TT adds over
dh (2x mode; the 1x tensor_reduce measured ~2x slower), softmax skips
max-subtraction (|scores| << 1), and V is pre-permuted to dh-major so the
exp-weighting broadcast stays in the DVE 2x packed mode. Per-tile aggregation
is a log-tree of packed adds in a separate product tile (in-place reuse of kvg
measurably regresses). Small copies that would run in DVE 2-port perf mode
(q_all, gp init) go on the Scalar engine: 2-port DVE ops stall 15-20us against
concurrent SWDGE descriptor-ring writes. LayerNorm computes rstd =
Exp(-0.5*Ln(var+eps)) so the Exp activation table is shared with attention.
The residual stream h_T is bf16. The whole schedule is chunk-pipelined (512
nodes): input-proj/LN/QKV per chunk up front, then per chunk attention ->
O-proj/LN2/FFN -> next-layer LN1/QKV -> per-chunk AllGather (AGQ=8) whose
output tile is addr_space=Shared (direct peer writes; the Local-output mesh
path measured 62-78us per 1MB-rank AG).

Measured dead ends (do not retry blindly): multi-offset indirect_dma_start
silently delivers only the first block per partition for non-consecutive rows;
AGQ=2(bf16+Local), ffn bufs=1, att bufs=3, GPAIR=4, SCAP=24-with-inplace,
gather-merging via tile pairing, fp8 AG + DRAM->DRAM upcast-DMA all regressed.
Also regressed: prep+trigger for EVERY gather (+1.07ms — per-pass trigger_dma
and sem-wait overhead dwarfs any emission win); moving the self edge out of
the gather into a local kv_loc DMA + per-tile DVE init (+15us — 9 extra small
DVE/ACT ops per tile cost more than the 5.7%% Q7 saving). prepare_only preps
issued BEFORE an AllGather invert RAW (Tile records the read against the
previous table writer -> NaN); issued after, they are correct but gain nothing
because gpsimd blocks on the collective completion. collective_compute is
gpsimd-only. Run-to-run variance measured ~5-15us on this kernel.
"""
import math
import numpy as np

CFG = dict(N=32768, D=256, H=8, DH=32, FFN=1024, L=2, C=8)
EPS = 1e-5
P = 128
SCAP = 25          # kvg slots per attention pass (self + edges)
KVBUFS = 2
AGQ = 1            # single whole-layer AllGather (Shared output)
NCHUNK = 512
T = 32


def _dims():
    N, D, C = CFG["N"], CFG["D"], CFG["C"]
    NS = N // C
    NCH = NS // NCHUNK
    return N, D, CFG["H"], CFG["DH"], CFG["FFN"], CFG["L"], C, NS, NCH


def _bf16(x):
    import ml_dtypes
    return np.asarray(x).astype(ml_dtypes.bfloat16)


def build_pass_plan(S_t, scap=SCAP):
    """One gather pass per tile; oversized tiles split. Grouped per chunk.

    Returns (plan, gcol): plan[g][p] = [(t, col0, ncols)] and gcol[g][p] =
    global pass-ordered column offset.
    """
    plan = []
    for g in range(T * P // NCHUNK):
        grp = []
        for t in range(g * 4, g * 4 + 4):
            ct = S_t[t] + 1
            k = 0
            while k < ct:
                n1 = ct - k if ct - k <= scap else (ct - k + 1) // 2
                n1 = min(n1, scap)
                grp.append([(t, k, n1)])
                k += n1
        plan.append(grp)
    gcol = []
    run = 0
    for grp in plan:
        offs = []
        for segs in grp:
            offs.append(run)
            run += sum(s[2] for s in segs)
        gcol.append(offs)
    return plan, gcol


def preprocess(edge_index):
    """Degree-sorted striped node permutation + pass-ordered gather tables."""
    N, D, H, DH, FFN_, L_, C, NS, NCH = _dims()
    src0 = np.asarray(edge_index[0], dtype=np.int64)
    dst0 = np.asarray(edge_index[1], dtype=np.int64)
    E = src0.shape[0]
    deg = np.bincount(dst0, minlength=N).astype(np.int64)

    # stripe assignment: rank r (by degree) -> core r%C, slot r//C
    order = np.argsort(deg, kind="stable")
    r = np.arange(N)
    c_of = np.empty(N, np.int64); l0_of = np.empty(N, np.int64)
    c_of[order] = r % C
    l0_of[order] = r // C
    tile_of = l0_of // P
    # within-tile descending degree resort
    keys = np.lexsort((l0_of, -deg, tile_of, c_of))
    pos = np.empty(N, np.int64)
    pos[keys] = np.arange(N)
    p_of = pos % P
    l_of = tile_of * P + p_of

    QCH = NS // AGQ
    trow_of = (l_of // QCH) * (C * QCH) + c_of * QCH + (l_of % QCH)

    # per-edge slot index (occurrence among in-edges of its dst)
    gd = c_of[dst0] * NS + l_of[dst0]
    eorder = np.argsort(gd, kind="stable")
    gds = gd[eorder]
    srcs = src0[eorder]
    bounds = np.searchsorted(gds, np.arange(N + 1))
    slot = np.arange(E) - bounds[gds]

    degmat = np.zeros((C, T, P), np.int64)
    degmat[c_of, tile_of, p_of] = deg
    S_t = tuple(int(s) for s in degmat.max(axis=2).max(axis=0))

    csC = np.concatenate([[0], np.cumsum(np.array(S_t) + 1)])   # tile-major cols
    SUMC = int(csC[-1])

    trow_node = np.zeros((C, T, P), np.int64)
    trow_node[c_of, tile_of, p_of] = trow_of

    # tile-major offsets: col 0 self, pads default to own row
    offs = np.empty((C, P, SUMC), np.int32)
    for t in range(T):
        offs[:, :, csC[t]:csC[t + 1]] = trow_node[:, t, :, None].astype(np.int32)
    dce = c_of[dst0[eorder]]
    dte = tile_of[dst0[eorder]]
    dpe = p_of[dst0[eorder]]
    col = csC[dte] + 1 + slot
    offs[dce, dpe, col] = trow_of[srcs].astype(np.int32)

    mask = np.zeros((C, P, SUMC), np.float32)
    for t in range(T):
        mask[:, :, csC[t]] = 1.0
        s_arr = np.arange(1, S_t[t] + 1)
        mask[:, :, csC[t] + 1:csC[t + 1]] = (degmat[:, t, :, None] >= s_arr).astype(np.float32)
    mask2 = (degmat > 0).astype(np.float32).transpose(0, 2, 1)  # [C, P, T]

    old_of = np.empty((C, NS), np.int64)
    old_of[c_of, l_of] = np.arange(N)

    # pass-ordered column permutation + per-pass 16-wrapped SWDGE idx blocks
    plan, _g = build_pass_plan(S_t)
    idx_parts = []
    perm = []
    for grp in plan:
        for segs in grp:
            cols = []
            for (t, col0, ncols) in segs:
                cols.extend(range(csC[t] + col0, csC[t] + col0 + ncols))
            perm.extend(cols)
            blk = offs[:, :, cols].transpose(0, 2, 1).reshape(C, len(cols) * P)
            w = blk.reshape(C, len(cols) * P // 16, 16).transpose(0, 2, 1)
            idx_parts.append(np.tile(w, (1, 8, 1)))
    idx16 = np.concatenate(idx_parts, axis=2).astype(np.int16)   # [C, 128, SUMC*8]
    mask_p = mask[:, :, perm]
    meta = (S_t,)
    return meta, idx16, mask_p, mask2, old_of, SUMC


def build_nc(meta, sumc, debug=False):
    import concourse.bacc as bacc
    import concourse.mybir as mybir
    import concourse.tile as tile
    from concourse import bass
    from concourse.masks import make_identity

    (S_t,) = meta
    N, D, H, DH, FFN, L, C, NS, NCH = _dims()
    TD = D
    fp32 = mybir.dt.float32
    bf16 = mybir.dt.bfloat16
    AF = mybir.ActivationFunctionType
    OP = mybir.AluOpType
    QCH = NS // AGQ
    plan, gcol = build_pass_plan(S_t)

    nc = bacc.Bacc("TRN2", target_bir_lowering=False, debug=False, num_devices=CFG["C"],
                   num_swdge_queues=4)

    x_t = nc.declare_dram_parameter("x_t", [2, P, NS], bf16, isOutput=False)
    w_in = nc.declare_dram_parameter("w_in", [2, P, D], bf16, isOutput=False)
    wqkv = nc.declare_dram_parameter("wqkv", [L, 2, P, 3 * TD], bf16, isOutput=False)
    wo = nc.declare_dram_parameter("wo", [L, 2, P, D], bf16, isOutput=False)
    w1 = nc.declare_dram_parameter("w1", [L, 2, P, FFN], bf16, isOutput=False)
    w2 = nc.declare_dram_parameter("w2", [L, 8, P, D], bf16, isOutput=False)
    idx_e = nc.declare_dram_parameter("idx16", [P, sumc * 8], mybir.dt.int16, isOutput=False)
    mask_e = nc.declare_dram_parameter("mask", [P, sumc], bf16, isOutput=False)
    mask2_e = nc.declare_dram_parameter("mask2", [P, T], bf16, isOutput=False)
    out_e = nc.declare_dram_parameter("out", [2, P, NS], bf16, isOutput=True)

    with nc.allow_low_precision("bf16 residual/score"), tile.TileContext(nc) as tc:
        with (
            tc.tile_pool(name="persist", bufs=1) as pp,
            tc.tile_pool(name="dram", bufs=1, space="DRAM") as dp,
            tc.tile_pool(name="kvgp", bufs=KVBUFS) as kp,
            tc.tile_pool(name="prodp", bufs=1) as rp,
            tc.tile_pool(name="att", bufs=2) as ap,
            tc.tile_pool(name="ln", bufs=2) as lp,
            tc.tile_pool(name="xin", bufs=2) as xp,
            tc.tile_pool(name="ffn", bufs=2) as fp,
            tc.tile_pool(name="ps_tr", bufs=2, space="PSUM") as ps_tr,
            tc.tile_pool(name="ps_ln", bufs=1, space="PSUM") as ps_ln,
            tc.tile_pool(name="ps_g", bufs=2, space="PSUM") as ps_g,
        ):
            from concourse import library_config
            nc.gpsimd.load_library(library_config.mlp)

            # ---- persistent SBUF ----
            h_T = pp.tile([P, 2, NS], bf16)
            act_T = pp.tile([P, 2, NS], bf16)
            w_in_sb = pp.tile([P, 2, D], bf16)
            wqkv_sb = pp.tile([P, L, 2, 3 * TD], bf16)
            wo_sb = pp.tile([P, L, 2, D], bf16)
            w1_sb = pp.tile([P, L, 2, FFN], bf16)
            w2_sb = pp.tile([P, L, 8, D], bf16)
            idx_sb = pp.tile([P, sumc * 8], mybir.dt.int16)
            mask_sb = pp.tile([P, sumc], bf16)
            mask2_sb = pp.tile([P, T], bf16)
            q_all = pp.tile([P, T, TD], bf16)
            ones_b = pp.tile([P, P], bf16)
            epsb = pp.tile([P, 1], fp32)
            ident = pp.tile([P, P], bf16)

            nc.sync.dma_start(w_in_sb[:], w_in[:].rearrange("c p d -> p c d"))
            nc.sync.dma_start(wqkv_sb[:], wqkv[:].rearrange("l c p d -> p l c d"))
            nc.sync.dma_start(wo_sb[:], wo[:].rearrange("l c p d -> p l c d"))
            nc.sync.dma_start(w1_sb[:], w1[:].rearrange("l c p d -> p l c d"))
            nc.sync.dma_start(w2_sb[:], w2[:].rearrange("l c p d -> p l c d"))
            nc.sync.dma_start(idx_sb[:], idx_e[:])
            nc.sync.dma_start(mask_sb[:], mask_e[:])
            nc.sync.dma_start(mask2_sb[:], mask2_e[:])
            nc.vector.memset(ones_b[:], 1.0 / D)
            nc.vector.memset(epsb[:], EPS)
            make_identity(nc, ident[:])

            kv_loc = [[dp.tile([QCH, 2 * TD], bf16, tag=f"kvloc{l}_{q}", name=f"kvloc{l}_{q}")
                       for q in range(AGQ)] for l in range(L)]
            kv_tab = [dp.tile([N, 2 * TD], bf16, tag=f"kvtab{l}", name=f"kvtab{l}",
                              addr_space="Shared")
                      for l in range(L)]

            # pre-touch kvg slots so pad partitions hold finite stale data
            for _i in range(KVBUFS):
                kvg_init = kp.tile([P, SCAP * 2 * TD], bf16, tag="kvg", name=f"kvg_init{_i}")
                nc.vector.memset(kvg_init[:], 0.0)

            def layernorm_chunk(src, dst, nchk):
                ns = slice(nchk * NCHUNK, (nchk + 1) * NCHUNK)
                mu_p = ps_ln.tile([P, NCHUNK], fp32, space="PSUM", tag="mu", name="mu_p")
                ex2_p = ps_ln.tile([P, NCHUNK], fp32, space="PSUM", tag="ex2", name="ex2_p")
                sq = lp.tile([P, 2, NCHUNK], bf16, tag="sq", name="sq")
                for c in range(2):
                    nc.scalar.activation(sq[:, c, :], src[:, c, ns], AF.Square)
                for c in range(2):
                    nc.tensor.matmul(mu_p[:], lhsT=ones_b[:], rhs=src[:, c, ns],
                                     start=(c == 0), stop=(c == 1))
                    nc.tensor.matmul(ex2_p[:], lhsT=ones_b[:], rhs=sq[:, c, :],
                                     start=(c == 0), stop=(c == 1))
                mu_sb = lp.tile([P, NCHUNK], bf16, tag="musb", name="mu_sb")
                nc.scalar.copy(mu_sb[:], mu_p[:])
                mu2b = lp.tile([P, NCHUNK], bf16, tag="mu2b", name="mu2b")
                nc.scalar.activation(mu2b[:], mu_p[:], AF.Square)
                mu2 = lp.tile([P, NCHUNK], bf16, tag="mu2", name="mu2")
                nc.vector.tensor_tensor(out=mu2[:], in0=ex2_p[:], in1=mu2b[:], op=OP.subtract)
                lg = lp.tile([P, NCHUNK], bf16, tag="sd", name="lg")
                nc.scalar.activation(lg[:], mu2[:], AF.Ln, bias=epsb[:])
                rstd = lp.tile([P, NCHUNK], bf16, tag="rstd", name="rstd")
                nc.scalar.activation(rstd[:], lg[:], AF.Exp, scale=-0.5)
                ms = lp.tile([P, NCHUNK], bf16, tag="ms", name="ms")
                nc.vector.tensor_tensor(out=ms[:], in0=mu_sb[:], in1=rstd[:], op=OP.mult)
                for c in range(2):
                    tmp = lp.tile([P, NCHUNK], bf16, tag="tmp", name="tmp")
                    nc.vector.tensor_tensor(out=tmp[:], in0=src[:, c, ns], in1=rstd[:], op=OP.mult)
                    nc.vector.tensor_tensor(out=dst[:, c, ns], in0=tmp[:], in1=ms[:], op=OP.subtract)

            def inputproj_chunk(nchk):
                ns = slice(nchk * NCHUNK, (nchk + 1) * NCHUNK)
                xin = xp.tile([P, 2, NCHUNK], bf16, tag="xin", name="xin")
                nc.sync.dma_start(xin[:], x_t[:, :, ns].rearrange("c p n -> p c n"))
                for co in range(2):
                    hp = ps_g.tile([P, NCHUNK], fp32, space="PSUM", tag="gemm", name="hp")
                    for ck in range(2):
                        nc.tensor.matmul(hp[:], lhsT=w_in_sb[:, ck, co * P:(co + 1) * P],
                                         rhs=xin[:, ck, :], start=(ck == 0), stop=(ck == 1))
                    nc.scalar.copy(h_T[:, co, ns], hp[:])

            def kv_tile(l, t):
                tsl = slice(t * P, (t + 1) * P)
                kv_b = ap.tile([P, 2 * TD], bf16, tag="kvb", name="kv_b")
                kv_p = ps_g.tile([P, 512], fp32, space="PSUM", tag="gemm", name="kv_p")
                for ck in range(2):
                    nc.tensor.matmul(kv_p[:], lhsT=act_T[:, ck, tsl],
                                     rhs=wqkv_sb[:, l, ck, 0:512],
                                     start=(ck == 0), stop=(ck == 1))
                nc.scalar.copy(kv_b[:], kv_p[:])
                q = t // (T // AGQ)
                r0 = (t % (T // AGQ)) * P
                nc.sync.dma_start(kv_loc[l][q][r0:r0 + P, :], kv_b[:])

            def q_tile(l, t):
                tsl = slice(t * P, (t + 1) * P)
                q_p = ps_g.tile([P, 512], fp32, space="PSUM", tag="gemm", name="q_p")
                for ck in range(2):
                    nc.tensor.matmul(q_p[:, 0:TD], lhsT=act_T[:, ck, tsl],
                                     rhs=wqkv_sb[:, l, ck, 512:768],
                                     start=(ck == 0), stop=(ck == 1))
                nc.scalar.copy(q_all[:, t, :], q_p[:, 0:TD])

            def allgather_chunk(l, q):
                rows = slice(q * C * QCH, (q + 1) * C * QCH)
                nc.gpsimd.collective_compute(
                    "AllGather", mybir.AluOpType.bypass,
                    ins=[kv_loc[l][q][:].rearrange("(u r) d -> u (r d)", u=1)],
                    outs=[kv_tab[l][rows, :].rearrange("(u r) d -> u (r d)", u=1)],
                    replica_groups=[list(range(C))],
                )

            qrr = [0]
            tstate = {}

            def issue_gather(l, segs, gc, prep_sem=None):
                nct = sum(s[2] for s in segs)
                kvg = kp.tile([P, SCAP * 2 * TD], bf16, tag="kvg", name="kvg")
                kv3 = kvg[:].rearrange("p (s d) -> p s d", d=2 * TD)
                qn = qrr[0] % 4
                nc.gpsimd.dma_gather(
                    kv3[:, 0:nct, :],
                    kv_tab[l][:],
                    idx_sb[:, gc * 8:(gc + nct) * 8],
                    nct * P, nct * P, 2 * TD,
                    single_packet=False,
                    queue_num=qn,
                    prepare_only=(prep_sem is not None),
                    sem=prep_sem,
                )
                qrr[0] += 1
                return kvg, qn

            NPREF = KVBUFS
            pref = []
            psem = [0]

            def prefetch_layer(l):
                flat = [(segs, gcol[g][pi]) for g in range(NCH)
                        for pi, segs in enumerate(plan[g])]
                for i in range(NPREF):
                    sem = nc.alloc_semaphore(f"pref{psem[0]}")
                    psem[0] += 1
                    kvg, qn = issue_gather(l, flat[i][0], flat[i][1], prep_sem=sem)
                    pref.append((kvg, qn, sem))

            def fire_prefetch():
                for _kvg, qn, _sem in pref:
                    nc.gpsimd.trigger_dma(count=None, queue_num=qn)

            def attention_pass(l, segs, gc, kvg=None, dsem=None):
                if kvg is None:
                    kvg, _ = issue_gather(l, segs, gc)
                if dsem is not None:
                    nc.vector.wait_ge(dsem, 16)
                kv3 = kvg[:].rearrange("p (s d) -> p s d", d=2 * TD)
                prod = rp.tile([P, SCAP, TD], bf16, tag="prod", name="prod")
                sc = ap.tile([P, SCAP * H], bf16, tag="sc", name="sc")
                exm = ap.tile([P, SCAP * H], bf16, tag="exm", name="exm")
                off = 0
                for (t, col0, ncols) in segs:
                    k3 = kv3[:, off:off + ncols, 0:TD]
                    v3 = kv3[:, off:off + ncols, TD:2 * TD]
                    CT = S_t[t] + 1
                    first, last = col0 == 0, col0 + ncols == CT
                    if first:
                        z_t = ap.tile([P, H], fp32, tag=f"z{t % 2}", name="z_t")
                        agg_t = ap.tile([P, TD], fp32, tag=f"agg{t % 2}", name="agg_t")
                        tstate[t] = (z_t, agg_t)
                    else:
                        z_t, agg_t = tstate[t]
                    # scores: prod = K * q_bcast ; tree-reduce over dh (2x TT adds)
                    pr = prod[:, off:off + ncols, :]
                    nc.vector.tensor_tensor(
                        out=pr[:], in0=k3[:],
                        in1=q_all[:, t, :].rearrange("p (s d) -> p s d", s=1).to_broadcast([P, ncols, TD]),
                        op=OP.mult)
                    pv = pr[:].rearrange("p s (h d) -> p (s h) d", h=H)
                    m = DH
                    while m > 2:
                        a = m // 2
                        nc.vector.tensor_tensor(
                            out=pv[:, :, 0:a], in0=pv[:, :, 0:a], in1=pv[:, :, a:m],
                            op=OP.add)
                        m = a
                    scs = sc[:, off * H:(off + ncols) * H]
                    exs = exm[:, off * H:(off + ncols) * H]
                    nc.vector.tensor_tensor(
                        out=scs[:], in0=pv[:, :, 0], in1=pv[:, :, 1], op=OP.add)
                    nc.scalar.activation(exs[:], scs[:], AF.Exp)
                    # mask pads (stale rows beyond per-core validity)
                    nc.vector.tensor_tensor(
                        out=exs[:].rearrange("p (s h) -> p s h", h=H),
                        in0=exs[:].rearrange("p (s h) -> p s h", h=H),
                        in1=mask_sb[:, gc + off:gc + off + ncols]
                            .rearrange("p (s u) -> p s u", u=1).to_broadcast([P, ncols, H]),
                        op=OP.mult)
                    # z partial
                    zp = z_t if first else ap.tile([P, H], fp32, tag="zp", name="zp")
                    nc.vector.tensor_reduce(
                        out=zp[:],
                        in_=exs[:].rearrange("p (s h) -> p h s", h=H),
                        axis=mybir.AxisListType.X, op=OP.add)
                    if not first:
                        nc.vector.tensor_tensor(out=z_t[:], in0=z_t[:], in1=zp[:], op=OP.add)
                    # weighted V: prod = V * exp_bcast (V is dh-major so h packs last)
                    nc.vector.tensor_tensor(
                        out=pr[:].rearrange("p s (u h) -> p s u h", h=H),
                        in0=v3[:].rearrange("p s (u h) -> p s u h", h=H),
                        in1=exs[:].rearrange("p (s u h) -> p s u h", h=H, u=1)
                            .to_broadcast([P, ncols, DH, H]),
                        op=OP.mult)
                    # tree-sum over slots -> agg partial
                    m = ncols
                    while m > 2:
                        a = m - m // 2
                        nc.vector.tensor_tensor(
                            out=pr[:, 0:m - a, :], in0=pr[:, 0:m - a, :],
                            in1=pr[:, a:m, :], op=OP.add)
                        m = a
                    gp = agg_t if first else ap.tile([P, TD], fp32, tag="gp", name="gp")
                    if m == 2:
                        nc.vector.tensor_tensor(out=gp[:], in0=pr[:, 0, :], in1=pr[:, 1, :], op=OP.add)
                    else:
                        nc.scalar.copy(gp[:], pr[:, 0, :])
                    if not first:
                        nc.vector.tensor_tensor(out=agg_t[:], in0=agg_t[:], in1=gp[:], op=OP.add)
                    if last:
                        finalize_tile(t, z_t, agg_t)
                    off += ncols

            def finalize_tile(t, z_t, agg_t):
                tsl = slice(t * P, (t + 1) * P)
                rz = ap.tile([P, H], fp32, tag="rz", name="rz")
                nc.vector.reciprocal(rz[:], z_t[:])
                rzm = ap.tile([P, H], bf16, tag="rzm", name="rzm")
                nc.vector.tensor_tensor(out=rzm[:], in0=rz[:],
                                        in1=mask2_sb[:, t:t + 1].to_broadcast([P, H]), op=OP.mult)
                att = ap.tile([P, TD], bf16, tag="att", name="att")
                nc.vector.tensor_tensor(
                    out=att[:].rearrange("p (u h) -> p u h", h=H),
                    in0=agg_t[:].rearrange("p (u h) -> p u h", h=H),
                    in1=rzm[:].rearrange("p (u h) -> p u h", u=1).to_broadcast([P, DH, H]),
                    op=OP.mult)
                trp = ps_tr.tile([P, 2, P], bf16, space="PSUM", tag="tr", name="trp")
                for c in range(2):
                    nc.tensor.transpose(trp[:, c, :], att[:, c * P:(c + 1) * P], ident[:])
                    nc.scalar.copy(act_T[:, c, tsl], trp[:, c, :])

            def oproj_chunk(l, nchk):
                ns = slice(nchk * NCHUNK, (nchk + 1) * NCHUNK)
                for co in range(2):
                    op_p = ps_g.tile([P, NCHUNK], fp32, space="PSUM", tag="gemm", name="op_p")
                    for ck in range(2):
                        nc.tensor.matmul(op_p[:], lhsT=wo_sb[:, l, ck, co * P:(co + 1) * P],
                                         rhs=act_T[:, ck, ns], start=(ck == 0), stop=(ck == 1))
                    op_b = lp.tile([P, NCHUNK], bf16, tag="resid", name="op_b")
                    nc.scalar.copy(op_b[:], op_p[:])
                    nc.vector.tensor_tensor(out=h_T[:, co, ns], in0=h_T[:, co, ns], in1=op_b[:], op=OP.add)

            def ffn_chunk(l, nchk):
                ns = slice(nchk * NCHUNK, (nchk + 1) * NCHUNK)
                h1 = fp.tile([P, 8, NCHUNK], bf16, tag="h1", name="h1")
                for m in range(8):
                    g1 = ps_g.tile([P, NCHUNK], fp32, space="PSUM", tag="gemm", name="g1")
                    for ck in range(2):
                        nc.tensor.matmul(g1[:], lhsT=w1_sb[:, l, ck, m * P:(m + 1) * P],
                                         rhs=act_T[:, ck, ns], start=(ck == 0), stop=(ck == 1))
                    nc.scalar.activation(h1[:, m, :], g1[:], AF.Gelu)
                for co in range(2):
                    g2 = ps_g.tile([P, NCHUNK], fp32, space="PSUM", tag="gemm", name="g2")
                    for ck in range(8):
                        nc.tensor.matmul(g2[:], lhsT=w2_sb[:, l, ck, co * P:(co + 1) * P],
                                         rhs=h1[:, ck, :], start=(ck == 0), stop=(ck == 7))
                    g2_b = lp.tile([P, NCHUNK], bf16, tag="resid", name="g2_b")
                    nc.scalar.copy(g2_b[:], g2[:])
                    nc.vector.tensor_tensor(out=h_T[:, co, ns], in0=h_T[:, co, ns], in1=g2_b[:], op=OP.add)

            TPC = NCHUNK // P  # tiles per node-chunk (4)

            # ---- layer 0 front: input-proj sweep, LN1+KV per chunk, AG,
            # then all Q GEMMs overlapping the AllGather ----
            for g in range(NCH):
                inputproj_chunk(g)
            for g in range(NCH):
                layernorm_chunk(h_T, act_T, g)
                for t in range(g * TPC, (g + 1) * TPC):
                    kv_tile(0, t)
            allgather_chunk(0, 0)
            prefetch_layer(0)
            fire_prefetch()
            for t in range(T):
                q_tile(0, t)

            for l in range(L):
                pcnt = [0]
                for g in range(NCH):
                    for pi, segs in enumerate(plan[g]):
                        if pcnt[0] < NPREF:
                            attention_pass(l, segs, gcol[g][pi], kvg=pref[0][0],
                                           dsem=pref[0][2])
                            pref.pop(0)
                        else:
                            attention_pass(l, segs, gcol[g][pi])
                        pcnt[0] += 1
                    oproj_chunk(l, g)
                    layernorm_chunk(h_T, act_T, g)
                    ffn_chunk(l, g)
                    if l + 1 < L:
                        layernorm_chunk(h_T, act_T, g)   # LN1 of next layer
                        for t in range(g * TPC, (g + 1) * TPC):
                            kv_tile(l + 1, t)
                        if g == NCH - 1:
                            allgather_chunk(l + 1, 0)
                            prefetch_layer(l + 1)
                            fire_prefetch()
                            for t2 in range(T):
                                q_tile(l + 1, t2)
                    else:
                        ns = slice(g * NCHUNK, (g + 1) * NCHUNK)
                        for c in range(2):
                            nc.sync.dma_start(out_e[c, :, ns], h_T[:, c, ns])

    nc.compile()
    return nc


def make_in_maps(x, edge_index, w_in, wq, wk, wv, wo, w1, w2):
    N, D, H, DH, FFN, L, C, NS, NCH = _dims()
    TD = D
    x = np.asarray(x, np.float32)
    meta, idx16, mask, mask2, old_of, SUMC = preprocess(edge_index)

    scale = 1.0 / math.sqrt(DH)
    wq_s = np.asarray(wq, np.float32) * scale
    wv_p = np.asarray(wv, np.float32).reshape(L, D, H, DH).transpose(0, 1, 3, 2).reshape(L, D, TD)
    wqkv_h = np.concatenate([np.asarray(wk, np.float32), wv_p, wq_s], axis=2)
    wqkv_h = _bf16(wqkv_h.reshape(L, 2, P, 3 * TD))
    w_in_h = _bf16(np.asarray(w_in, np.float32).reshape(2, P, D))
    wo_p = np.asarray(wo, np.float32).reshape(L, H, DH, D).transpose(0, 2, 1, 3).reshape(L, TD, D)
    wo_h = _bf16(wo_p.reshape(L, 2, P, D))
    w1_h = _bf16(np.asarray(w1, np.float32).reshape(L, 2, P, FFN))
    w2_h = _bf16(np.asarray(w2, np.float32).reshape(L, 8, P, D))

    in_maps = []
    for c in range(C):
        xs = _bf16(x[old_of[c], :].T)
        in_maps.append({
            "x_t": np.ascontiguousarray(xs.reshape(2, P, NS)),
            "w_in": w_in_h, "wqkv": wqkv_h, "wo": wo_h, "w1": w1_h, "w2": w2_h,
            "idx16": np.ascontiguousarray(idx16[c]),
            "mask": _bf16(mask[c]), "mask2": _bf16(mask2[c]),
        })
    return meta, SUMC, old_of, in_maps


def assemble_out(results, old_of):
    N, D, H, DH, FFN, L, C, NS, NCH = _dims()
    out = np.empty((N, D), np.float32)
    for c in range(C):
        o = np.asarray(results[c]["out"], np.float32).reshape(2 * P, NS)
        out[old_of[c], :] = o.T
    return out


_BUILD_CACHE = {}


def _get_nc(meta, sumc):
    key = (meta, sumc)
    if key not in _BUILD_CACHE:
        _BUILD_CACHE[key] = build_nc(meta, sumc)
    return _BUILD_CACHE[key]


def kernel(x, edge_index, w_in, b_in, ln1_g, ln1_b, ln2_g, ln2_b,
           wq, bq, wk, bk, wv, bv, wo, bo, w1, b1, w2, b2, _trace=False):
    from concourse.bass_utils import run_bass_kernel_spmd

    for b in (b_in, bq, bk, bv, bo, b1, b2, ln1_b, ln2_b):
        assert np.abs(np.asarray(b)).max() == 0.0, "nonzero bias unsupported"
    for g in (ln1_g, ln2_g):
        assert np.abs(np.asarray(g) - 1.0).max() == 0.0, "non-unit LN gamma unsupported"

    meta, sumc, old_of, in_maps = make_in_maps(x, edge_index, w_in, wq, wk, wv, wo, w1, w2)
    nc = _get_nc(meta, sumc)
    res = run_bass_kernel_spmd(nc, in_maps, core_ids=list(range(CFG["C"])), trace=_trace)
    if _trace:
        kernel._last_result = res
    return assemble_out(res.results, old_of)


# revision 25
# speedup vs baseline: 1.1152x; 1.1152x over previous
"""AllostericGNN Trainium2 kernel (8 NeuronCores, SPMD) — slot-major attention.

Strategy: shard nodes (and their in-edges, grouped by dst) across 8 cores.
Nodes are degree-sorted and striped across cores so all cores see an identical
degree profile (the compiled program is shared); the inverse permutation is
applied on the host at output assembly. Per 128-dst tile, in-edges live in
"slots" along the free axis of the dst partition: the self edge is slot 0 and
edge slots follow, so ONE SWDGE dma_gather per tile-pass covers self+edges
(s-major 16-wrapped int16 indices, single_packet=False, 4 SWDGE queues
round-robin; Q7 descriptor emission is ~5.3ns/idx flat — merging gathers
bought nothing and fp8 wire + upcast-DMA stole SDMA bandwidth from gathers:
both measured dead ends). Pad slots point at valid self rows and are masked
after exp. Scores are a broadcast multiply + log-tree of packed TT adds over
dh (2x mode; the 1x tensor_reduce measured ~2x slower), softmax skips
max-subtraction (|scores| << 1), and V is pre-permuted to dh-major so the
exp-weighting broadcast stays in the DVE 2x packed mode. Per-tile aggregation
is a log-tree of packed adds in a separate product tile (in-place reuse of kvg
measurably regresses). Small copies that would run in DVE 2-port perf mode
(q_all, gp init) go on the Scalar engine: 2-port DVE ops stall 15-20us against
concurrent SWDGE descriptor-ring writes. LayerNorm computes rstd =
Exp(-0.5*Ln(var+eps)) so the Exp activation table is shared with attention.
The residual stream h_T is bf16. The whole schedule is chunk-pipelined (512
nodes): input-proj/LN/QKV per chunk up front, then per chunk attention ->
O-proj/LN2/FFN -> next-layer LN1/QKV -> per-chunk AllGather (AGQ=8) whose
output tile is addr_space=Shared (direct peer writes; the Local-output mesh
path measured 62-78us per 1MB-rank AG).

Measured dead ends (do not retry blindly): multi-offset indirect_dma_start
silently delivers only the first block per partition for non-consecutive rows;
AGQ=2(bf16+Local), ffn bufs=1, att bufs=3, GPAIR=4, SCAP=24-with-inplace,
gather-merging via tile pairing, fp8 AG + DRAM->DRAM upcast-DMA all regressed.
Also regressed: prep+trigger for EVERY gather (+1.07ms — per-pass trigger_dma
and sem-wait overhead dwarfs any emission win); moving the self edge out of
the gather into a local kv_loc DMA + per-tile DVE init (+15us — 9 extra small
DVE/ACT ops per tile cost more than the 5.7%% Q7 saving). prepare_only preps
issued BEFORE an AllGather invert RAW (Tile records the read against the
previous table writer -> NaN); issued after, they are correct but gain nothing
because gpsimd blocks on the collective completion. collective_compute is
gpsimd-only. Run-to-run variance measured ~5-15us on this kernel.
"""
import math
import numpy as np

CFG = dict(N=32768, D=256, H=8, DH=32, FFN=1024, L=2, C=8)
EPS = 1e-5
P = 128
SCAP = 20          # kvg slots per attention pass (self + edges)
KVBUFS = 3
AGQ = 1            # single whole-layer AllGather (Shared output)
NCHUNK = 512
T = 32


def _dims():
    N, D, C = CFG["N"], CFG["D"], CFG["C"]
    NS = N // C
    NCH = NS // NCHUNK
    return N, D, CFG["H"], CFG["DH"], CFG["FFN"], CFG["L"], C, NS, NCH


def _bf16(x):
    import ml_dtypes
    return np.asarray(x).astype(ml_dtypes.bfloat16)


def build_pass_plan(S_t, scap=SCAP):
    """One gather pass per tile; oversized tiles split. Grouped per chunk.

    Returns (plan, gcol): plan[g][p] = [(t, col0, ncols)] and gcol[g][p] =
    global pass-ordered column offset.
    """
    plan = []
    for g in range(T * P // NCHUNK):
        grp = []
        for t in range(g * 4, g * 4 + 4):
            ct = S_t[t] + 1
            k = 0
            while k < ct:
                n1 = ct - k if ct - k <= scap else (ct - k + 1) // 2
                n1 = min(n1, scap)
                grp.append([(t, k, n1)])
                k += n1
        plan.append(grp)
    gcol = []
    run = 0
    for grp in plan:
        offs = []
        for segs in grp:
            offs.append(run)
            run += sum(s[2] for s in segs)
        gcol.append(offs)
    return plan, gcol


def preprocess(edge_index):
    """Degree-sorted striped node permutation + pass-ordered gather tables."""
    N, D, H, DH, FFN_, L_, C, NS, NCH = _dims()
    src0 = np.asarray(edge_index[0], dtype=np.int64)
    dst0 = np.asarray(edge_index[1], dtype=np.int64)
    E = src0.shape[0]
    deg = np.bincount(dst0, minlength=N).astype(np.int64)

    # stripe assignment: rank r (by degree) -> core r%C, slot r//C
    order = np.argsort(deg, kind="stable")
    r = np.arange(N)
    c_of = np.empty(N, np.int64); l0_of = np.empty(N, np.int64)
    c_of[order] = r % C
    l0_of[order] = r // C
    tile_of = l0_of // P
    # within-tile descending degree resort
    keys = np.lexsort((l0_of, -deg, tile_of, c_of))
    pos = np.empty(N, np.int64)
    pos[keys] = np.arange(N)
    p_of = pos % P
    l_of = tile_of * P + p_of

    QCH = NS // AGQ
    trow_of = (l_of // QCH) * (C * QCH) + c_of * QCH + (l_of % QCH)

    # per-edge slot index (occurrence among in-edges of its dst)
    gd = c_of[dst0] * NS + l_of[dst0]
    eorder = np.argsort(gd, kind="stable")
    gds = gd[eorder]
    srcs = src0[eorder]
    bounds = np.searchsorted(gds, np.arange(N + 1))
    slot = np.arange(E) - bounds[gds]

    degmat = np.zeros((C, T, P), np.int64)
    degmat[c_of, tile_of, p_of] = deg
    S_t = tuple(int(s) for s in degmat.max(axis=2).max(axis=0))

    csC = np.concatenate([[0], np.cumsum(np.array(S_t) + 1)])   # tile-major cols
    SUMC = int(csC[-1])

    trow_node = np.zeros((C, T, P), np.int64)
    trow_node[c_of, tile_of, p_of] = trow_of

    # tile-major offsets: col 0 self, pads default to own row
    offs = np.empty((C, P, SUMC), np.int32)
    for t in range(T):
        offs[:, :, csC[t]:csC[t + 1]] = trow_node[:, t, :, None].astype(np.int32)
    dce = c_of[dst0[eorder]]
    dte = tile_of[dst0[eorder]]
    dpe = p_of[dst0[eorder]]
    col = csC[dte] + 1 + slot
    offs[dce, dpe, col] = trow_of[srcs].astype(np.int32)

    mask = np.zeros((C, P, SUMC), np.float32)
    for t in range(T):
        mask[:, :, csC[t]] = 1.0
        s_arr = np.arange(1, S_t[t] + 1)
        mask[:, :, csC[t] + 1:csC[t + 1]] = (degmat[:, t, :, None] >= s_arr).astype(np.float32)
    mask2 = (degmat > 0).astype(np.float32).transpose(0, 2, 1)  # [C, P, T]

    old_of = np.empty((C, NS), np.int64)
    old_of[c_of, l_of] = np.arange(N)

    # pass-ordered column permutation + per-pass 16-wrapped SWDGE idx blocks
    plan, _g = build_pass_plan(S_t)
    idx_parts = []
    perm = []
    for grp in plan:
        for segs in grp:
            cols = []
            for (t, col0, ncols) in segs:
                cols.extend(range(csC[t] + col0, csC[t] + col0 + ncols))
            perm.extend(cols)
            blk = offs[:, :, cols].transpose(0, 2, 1).reshape(C, len(cols) * P)
            w = blk.reshape(C, len(cols) * P // 16, 16).transpose(0, 2, 1)
            idx_parts.append(np.tile(w, (1, 8, 1)))
    idx16 = np.concatenate(idx_parts, axis=2).astype(np.int16)   # [C, 128, SUMC*8]
    mask_p = mask[:, :, perm]
    meta = (S_t,)
    return meta, idx16, mask_p, mask2, old_of, SUMC


def build_nc(meta, sumc, debug=False):
    import concourse.bacc as bacc
    import concourse.mybir as mybir
    import concourse.tile as tile
    from concourse import bass
    from concourse.masks import make_identity

    (S_t,) = meta
    N, D, H, DH, FFN, L, C, NS, NCH = _dims()
    TD = D
    fp32 = mybir.dt.float32
    bf16 = mybir.dt.bfloat16
    AF = mybir.ActivationFunctionType
    OP = mybir.AluOpType
    QCH = NS // AGQ
    plan, gcol = build_pass_plan(S_t)

    nc = bacc.Bacc("TRN2", target_bir_lowering=False, debug=False, num_devices=CFG["C"],
                   num_swdge_queues=4)

    x_t = nc.declare_dram_parameter("x_t", [2, P, NS], bf16, isOutput=False)
    w_in = nc.declare_dram_parameter("w_in", [2, P, D], bf16, isOutput=False)
    wqkv = nc.declare_dram_parameter("wqkv", [L, 2, P, 3 * TD], bf16, isOutput=False)
    wo = nc.declare_dram_parameter("wo", [L, 2, P, D], bf16, isOutput=False)
    w1 = nc.declare_dram_parameter("w1", [L, 2, P, FFN], bf16, isOutput=False)
    w2 = nc.declare_dram_parameter("w2", [L, 8, P, D], bf16, isOutput=False)
    idx_e = nc.declare_dram_parameter("idx16", [P, sumc * 8], mybir.dt.int16, isOutput=False)
    mask_e = nc.declare_dram_parameter("mask", [P, sumc], bf16, isOutput=False)
    mask2_e = nc.declare_dram_parameter("mask2", [P, T], bf16, isOutput=False)
    out_e = nc.declare_dram_parameter("out", [2, P, NS], bf16, isOutput=True)

    with nc.allow_low_precision("bf16 residual/score"), tile.TileContext(nc) as tc:
        with (
            tc.tile_pool(name="persist", bufs=1) as pp,
            tc.tile_pool(name="dram", bufs=1, space="DRAM") as dp,
            tc.tile_pool(name="kvgp", bufs=KVBUFS) as kp,
            tc.tile_pool(name="prodp", bufs=1) as rp,
            tc.tile_pool(name="att", bufs=2) as ap,
            tc.tile_pool(name="ln", bufs=2) as lp,
            tc.tile_pool(name="xin", bufs=2) as xp,
            tc.tile_pool(name="ffn", bufs=2) as fp,
            tc.tile_pool(name="ps_tr", bufs=2, space="PSUM") as ps_tr,
            tc.tile_pool(name="ps_ln", bufs=1, space="PSUM") as ps_ln,
            tc.tile_pool(name="ps_g", bufs=2, space="PSUM") as ps_g,
        ):
            from concourse import library_config
            nc.gpsimd.load_library(library_config.mlp)

            # ---- persistent SBUF ----
            h_T = pp.tile([P, 2, NS], bf16)
            act_T = pp.tile([P, 2, NS], bf16)
            w_in_sb = pp.tile([P, 2, D], bf16)
            wqkv_sb = pp.tile([P, L, 2, 3 * TD], bf16)
            wo_sb = pp.tile([P, L, 2, D], bf16)
            w1_sb = pp.tile([P, L, 2, FFN], bf16)
            w2_sb = pp.tile([P, L, 8, D], bf16)
            idx_sb = pp.tile([P, sumc * 8], mybir.dt.int16)
            mask_sb = pp.tile([P, sumc], bf16)
            mask2_sb = pp.tile([P, T], bf16)
            q_all = pp.tile([P, T, TD], bf16)
            ones_b = pp.tile([P, P], bf16)
            epsb = pp.tile([P, 1], fp32)
            ident = pp.tile([P, P], bf16)

            nc.sync.dma_start(w_in_sb[:], w_in[:].rearrange("c p d -> p c d"))
            nc.sync.dma_start(wqkv_sb[:], wqkv[:].rearrange("l c p d -> p l c d"))
            nc.sync.dma_start(wo_sb[:], wo[:].rearrange("l c p d -> p l c d"))
            nc.sync.dma_start(w1_sb[:], w1[:].rearrange("l c p d -> p l c d"))
            nc.sync.dma_start(w2_sb[:], w2[:].rearrange("l c p d -> p l c d"))
            nc.sync.dma_start(idx_sb[:], idx_e[:])
            nc.sync.dma_start(mask_sb[:], mask_e[:])
            nc.sync.dma_start(mask2_sb[:], mask2_e[:])
            nc.vector.memset(ones_b[:], 1.0 / D)
            nc.vector.memset(epsb[:], EPS)
            make_identity(nc, ident[:])

            kv_loc = [[dp.tile([QCH, 2 * TD], bf16, tag=f"kvloc{l}_{q}", name=f"kvloc{l}_{q}")
                       for q in range(AGQ)] for l in range(L)]
            kv_tab = [dp.tile([N, 2 * TD], bf16, tag=f"kvtab{l}", name=f"kvtab{l}",
                              addr_space="Shared")
                      for l in range(L)]

            # pre-touch kvg slots so pad partitions hold finite stale data
            for _i in range(KVBUFS):
                kvg_init = kp.tile([P, SCAP * 2 * TD], bf16, tag="kvg", name=f"kvg_init{_i}")
                nc.vector.memset(kvg_init[:], 0.0)

            def layernorm_chunk(src, dst, nchk):
                ns = slice(nchk * NCHUNK, (nchk + 1) * NCHUNK)
                mu_p = ps_ln.tile([P, NCHUNK], fp32, space="PSUM", tag="mu", name="mu_p")
                ex2_p = ps_ln.tile([P, NCHUNK], fp32, space="PSUM", tag="ex2", name="ex2_p")
                sq = lp.tile([P, 2, NCHUNK], bf16, tag="sq", name="sq")
                for c in range(2):
                    nc.scalar.activation(sq[:, c, :], src[:, c, ns], AF.Square)
                for c in range(2):
                    nc.tensor.matmul(mu_p[:], lhsT=ones_b[:], rhs=src[:, c, ns],
                                     start=(c == 0), stop=(c == 1))
                    nc.tensor.matmul(ex2_p[:], lhsT=ones_b[:], rhs=sq[:, c, :],
                                     start=(c == 0), stop=(c == 1))
                mu_sb = lp.tile([P, NCHUNK], bf16, tag="musb", name="mu_sb")
                nc.scalar.copy(mu_sb[:], mu_p[:])
                mu2b = lp.tile([P, NCHUNK], bf16, tag="mu2b", name="mu2b")
                nc.scalar.activation(mu2b[:], mu_p[:], AF.Square)
                mu2 = lp.tile([P, NCHUNK], bf16, tag="mu2", name="mu2")
                nc.vector.tensor_tensor(out=mu2[:], in0=ex2_p[:], in1=mu2b[:], op=OP.subtract)
                lg = lp.tile([P, NCHUNK], bf16, tag="sd", name="lg")
                nc.scalar.activation(lg[:], mu2[:], AF.Ln, bias=epsb[:])
                rstd = lp.tile([P, NCHUNK], bf16, tag="rstd", name="rstd")
                nc.scalar.activation(rstd[:], lg[:], AF.Exp, scale=-0.5)
                ms = lp.tile([P, NCHUNK], bf16, tag="ms", name="ms")
                nc.vector.tensor_tensor(out=ms[:], in0=mu_sb[:], in1=rstd[:], op=OP.mult)
                for c in range(2):
                    tmp = lp.tile([P, NCHUNK], bf16, tag="tmp", name="tmp")
                    nc.vector.tensor_tensor(out=tmp[:], in0=src[:, c, ns], in1=rstd[:], op=OP.mult)
                    nc.vector.tensor_tensor(out=dst[:, c, ns], in0=tmp[:], in1=ms[:], op=OP.subtract)

            def inputproj_chunk(nchk):
                ns = slice(nchk * NCHUNK, (nchk + 1) * NCHUNK)
                xin = xp.tile([P, 2, NCHUNK], bf16, tag="xin", name="xin")
                nc.sync.dma_start(xin[:], x_t[:, :, ns].rearrange("c p n -> p c n"))
                for co in range(2):
                    hp = ps_g.tile([P, NCHUNK], fp32, space="PSUM", tag="gemm", name="hp")
                    for ck in range(2):
                        nc.tensor.matmul(hp[:], lhsT=w_in_sb[:, ck, co * P:(co + 1) * P],
                                         rhs=xin[:, ck, :], start=(ck == 0), stop=(ck == 1))
                    nc.scalar.copy(h_T[:, co, ns], hp[:])

            def kv_tile(l, t):
                tsl = slice(t * P, (t + 1) * P)
                kv_b = ap.tile([P, 2 * TD], bf16, tag="kvb", name="kv_b")
                kv_p = ps_g.tile([P, 512], fp32, space="PSUM", tag="gemm", name="kv_p")
                for ck in range(2):
                    nc.tensor.matmul(kv_p[:], lhsT=act_T[:, ck, tsl],
                                     rhs=wqkv_sb[:, l, ck, 0:512],
                                     start=(ck == 0), stop=(ck == 1))
                nc.scalar.copy(kv_b[:], kv_p[:])
                q = t // (T // AGQ)
                r0 = (t % (T // AGQ)) * P
                nc.sync.dma_start(kv_loc[l][q][r0:r0 + P, :], kv_b[:])

            def q_tile(l, t):
                tsl = slice(t * P, (t + 1) * P)
                q_p = ps_g.tile([P, 512], fp32, space="PSUM", tag="gemm", name="q_p")
                for ck in range(2):
                    nc.tensor.matmul(q_p[:, 0:TD], lhsT=act_T[:, ck, tsl],
                                     rhs=wqkv_sb[:, l, ck, 512:768],
                                     start=(ck == 0), stop=(ck == 1))
                nc.scalar.copy(q_all[:, t, :], q_p[:, 0:TD])

            def allgather_chunk(l, q):
                rows = slice(q * C * QCH, (q + 1) * C * QCH)
                nc.gpsimd.collective_compute(
                    "AllGather", mybir.AluOpType.bypass,
                    ins=[kv_loc[l][q][:].rearrange("(u r) d -> u (r d)", u=1)],
                    outs=[kv_tab[l][rows, :].rearrange("(u r) d -> u (r d)", u=1)],
                    replica_groups=[list(range(C))],
                )

            qrr = [0]
            tstate = {}

            def issue_gather(l, segs, gc, prep_sem=None):
                nct = sum(s[2] for s in segs)
                kvg = kp.tile([P, SCAP * 2 * TD], bf16, tag="kvg", name="kvg")
                kv3 = kvg[:].rearrange("p (s d) -> p s d", d=2 * TD)
                qn = qrr[0] % 4
                nc.gpsimd.dma_gather(
                    kv3[:, 0:nct, :],
                    kv_tab[l][:],
                    idx_sb[:, gc * 8:(gc + nct) * 8],
                    nct * P, nct * P, 2 * TD,
                    single_packet=False,
                    queue_num=qn,
                    prepare_only=(prep_sem is not None),
                    sem=prep_sem,
                )
                qrr[0] += 1
                return kvg, qn

            NPREF = KVBUFS
            pref = []
            psem = [0]

            def prefetch_layer(l):
                flat = [(segs, gcol[g][pi]) for g in range(NCH)
                        for pi, segs in enumerate(plan[g])]
                for i in range(NPREF):
                    sem = nc.alloc_semaphore(f"pref{psem[0]}")
                    psem[0] += 1
                    kvg, qn = issue_gather(l, flat[i][0], flat[i][1], prep_sem=sem)
                    pref.append((kvg, qn, sem))

            def fire_prefetch():
                for _kvg, qn, _sem in pref:
                    nc.gpsimd.trigger_dma(count=None, queue_num=qn)

            def attention_pass(l, segs, gc, kvg=None, dsem=None):
                if kvg is None:
                    kvg, _ = issue_gather(l, segs, gc)
                if dsem is not None:
                    nc.vector.wait_ge(dsem, 16)
                kv3 = kvg[:].rearrange("p (s d) -> p s d", d=2 * TD)
                prod = rp.tile([P, SCAP, TD], bf16, tag="prod", name="prod")
                sc = ap.tile([P, SCAP * H], bf16, tag="sc", name="sc")
                exm = ap.tile([P, SCAP * H], bf16, tag="exm", name="exm")
                off = 0
                for (t, col0, ncols) in segs:
                    k3 = kv3[:, off:off + ncols, 0:TD]
                    v3 = kv3[:, off:off + ncols, TD:2 * TD]
                    CT = S_t[t] + 1
                    first, last = col0 == 0, col0 + ncols == CT
                    if first:
                        z_t = ap.tile([P, H], fp32, tag=f"z{t % 2}", name="z_t")
                        agg_t = ap.tile([P, TD], fp32, tag=f"agg{t % 2}", name="agg_t")
                        tstate[t] = (z_t, agg_t)
                    else:
                        z_t, agg_t = tstate[t]
                    # scores: prod = K * q_bcast ; tree-reduce over dh (2x TT adds)
                    pr = prod[:, off:off + ncols, :]
                    nc.vector.tensor_tensor(
                        out=pr[:], in0=k3[:],
                        in1=q_all[:, t, :].rearrange("p (s d) -> p s d", s=1).to_broadcast([P, ncols, TD]),
                        op=OP.mult)
                    pv = pr[:].rearrange("p s (h d) -> p (s h) d", h=H)
                    m = DH
                    while m > 2:
                        a = m // 2
                        nc.vector.tensor_tensor(
                            out=pv[:, :, 0:a], in0=pv[:, :, 0:a], in1=pv[:, :, a:m],
                            op=OP.add)
                        m = a
                    scs = sc[:, off * H:(off + ncols) * H]
                    exs = exm[:, off * H:(off + ncols) * H]
                    nc.vector.tensor_tensor(
                        out=scs[:], in0=pv[:, :, 0], in1=pv[:, :, 1], op=OP.add)
                    nc.scalar.activation(exs[:], scs[:], AF.Exp)
                    # mask pads (stale rows beyond per-core validity)
                    nc.vector.tensor_tensor(
                        out=exs[:].rearrange("p (s h) -> p s h", h=H),
                        in0=exs[:].rearrange("p (s h) -> p s h", h=H),
                        in1=mask_sb[:, gc + off:gc + off + ncols]
                            .rearrange("p (s u) -> p s u", u=1).to_broadcast([P, ncols, H]),
                        op=OP.mult)
                    # z partial
                    zp = z_t if first else ap.tile([P, H], fp32, tag="zp", name="zp")
                    nc.vector.tensor_reduce(
                        out=zp[:],
                        in_=exs[:].rearrange("p (s h) -> p h s", h=H),
                        axis=mybir.AxisListType.X, op=OP.add)
                    if not first:
                        nc.vector.tensor_tensor(out=z_t[:], in0=z_t[:], in1=zp[:], op=OP.add)
                    # weighted V: prod = V * exp_bcast (V is dh-major so h packs last)
                    nc.vector.tensor_tensor(
                        out=pr[:].rearrange("p s (u h) -> p s u h", h=H),
                        in0=v3[:].rearrange("p s (u h) -> p s u h", h=H),
                        in1=exs[:].rearrange("p (s u h) -> p s u h", h=H, u=1)
                            .to_broadcast([P, ncols, DH, H]),
                        op=OP.mult)
                    # tree-sum over slots -> agg partial
                    m = ncols
                    while m > 2:
                        a = m - m // 2
                        nc.vector.tensor_tensor(
                            out=pr[:, 0:m - a, :], in0=pr[:, 0:m - a, :],
                            in1=pr[:, a:m, :], op=OP.add)
                        m = a
                    gp = agg_t if first else ap.tile([P, TD], fp32, tag="gp", name="gp")
                    if m == 2:
                        nc.vector.tensor_tensor(out=gp[:], in0=pr[:, 0, :], in1=pr[:, 1, :], op=OP.add)
                    else:
                        nc.scalar.copy(gp[:], pr[:, 0, :])
                    if not first:
                        nc.vector.tensor_tensor(out=agg_t[:], in0=agg_t[:], in1=gp[:], op=OP.add)
                    if last:
                        finalize_tile(t, z_t, agg_t)
                    off += ncols

            def finalize_tile(t, z_t, agg_t):
                tsl = slice(t * P, (t + 1) * P)
                rz = ap.tile([P, H], fp32, tag="rz", name="rz")
                nc.vector.reciprocal(rz[:], z_t[:])
                rzm = ap.tile([P, H], bf16, tag="rzm", name="rzm")
                nc.vector.tensor_tensor(out=rzm[:], in0=rz[:],
                                        in1=mask2_sb[:, t:t + 1].to_broadcast([P, H]), op=OP.mult)
                att = ap.tile([P, TD], bf16, tag="att", name="att")
                nc.vector.tensor_tensor(
                    out=att[:].rearrange("p (u h) -> p u h", h=H),
                    in0=agg_t[:].rearrange("p (u h) -> p u h", h=H),
                    in1=rzm[:].rearrange("p (u h) -> p u h", u=1).to_broadcast([P, DH, H]),
                    op=OP.mult)
                trp = ps_tr.tile([P, 2, P], bf16, space="PSUM", tag="tr", name="trp")
                for c in range(2):
                    nc.tensor.transpose(trp[:, c, :], att[:, c * P:(c + 1) * P], ident[:])
                    nc.scalar.copy(act_T[:, c, tsl], trp[:, c, :])

            def oproj_chunk(l, nchk):
                ns = slice(nchk * NCHUNK, (nchk + 1) * NCHUNK)
                for co in range(2):
                    op_p = ps_g.tile([P, NCHUNK], fp32, space="PSUM", tag="gemm", name="op_p")
                    for ck in range(2):
                        nc.tensor.matmul(op_p[:], lhsT=wo_sb[:, l, ck, co * P:(co + 1) * P],
                                         rhs=act_T[:, ck, ns], start=(ck == 0), stop=(ck == 1))
                    op_b = lp.tile([P, NCHUNK], bf16, tag="resid", name="op_b")
                    nc.scalar.copy(op_b[:], op_p[:])
                    nc.vector.tensor_tensor(out=h_T[:, co, ns], in0=h_T[:, co, ns], in1=op_b[:], op=OP.add)

            def ffn_chunk(l, nchk):
                ns = slice(nchk * NCHUNK, (nchk + 1) * NCHUNK)
                h1 = fp.tile([P, 8, NCHUNK], bf16, tag="h1", name="h1")
                for m in range(8):
                    g1 = ps_g.tile([P, NCHUNK], fp32, space="PSUM", tag="gemm", name="g1")
                    for ck in range(2):
                        nc.tensor.matmul(g1[:], lhsT=w1_sb[:, l, ck, m * P:(m + 1) * P],
                                         rhs=act_T[:, ck, ns], start=(ck == 0), stop=(ck == 1))
                    nc.scalar.activation(h1[:, m, :], g1[:], AF.Gelu)
                for co in range(2):
                    g2 = ps_g.tile([P, NCHUNK], fp32, space="PSUM", tag="gemm", name="g2")
                    for ck in range(8):
                        nc.tensor.matmul(g2[:], lhsT=w2_sb[:, l, ck, co * P:(co + 1) * P],
                                         rhs=h1[:, ck, :], start=(ck == 0), stop=(ck == 7))
                    g2_b = lp.tile([P, NCHUNK], bf16, tag="resid", name="g2_b")
                    nc.scalar.copy(g2_b[:], g2[:])
                    nc.vector.tensor_tensor(out=h_T[:, co, ns], in0=h_T[:, co, ns], in1=g2_b[:], op=OP.add)

            TPC = NCHUNK // P  # tiles per node-chunk (4)

            # ---- layer 0 front: input-proj sweep, LN1+KV per chunk, AG,
            # then all Q GEMMs overlapping the AllGather ----
            for g in range(NCH):
                inputproj_chunk(g)
            for g in range(NCH):
                layernorm_chunk(h_T, act_T, g)
                for t in range(g * TPC, (g + 1) * TPC):
                    kv_tile(0, t)
            allgather_chunk(0, 0)
            prefetch_layer(0)
            fire_prefetch()
            for t in range(T):
                q_tile(0, t)

            for l in range(L):
                pcnt = [0]
                for g in range(NCH):
                    for pi, segs in enumerate(plan[g]):
                        if pcnt[0] < NPREF:
                            attention_pass(l, segs, gcol[g][pi], kvg=pref[0][0],
                                           dsem=pref[0][2])
                            pref.pop(0)
                        else:
                            attention_pass(l, segs, gcol[g][pi])
                        pcnt[0] += 1
                    oproj_chunk(l, g)
                    layernorm_chunk(h_T, act_T, g)
                    ffn_chunk(l, g)
                    if l + 1 < L:
                        layernorm_chunk(h_T, act_T, g)   # LN1 of next layer
                        for t in range(g * TPC, (g + 1) * TPC):
                            kv_tile(l + 1, t)
                        if g == NCH - 1:
                            allgather_chunk(l + 1, 0)
                            prefetch_layer(l + 1)
                            fire_prefetch()
                            for t2 in range(T):
                                q_tile(l + 1, t2)
                    else:
                        ns = slice(g * NCHUNK, (g + 1) * NCHUNK)
                        for c in range(2):
                            nc.sync.dma_start(out_e[c, :, ns], h_T[:, c, ns])

    nc.compile()
    return nc


def make_in_maps(x, edge_index, w_in, wq, wk, wv, wo, w1, w2):
    N, D, H, DH, FFN, L, C, NS, NCH = _dims()
    TD = D
    x = np.asarray(x, np.float32)
    meta, idx16, mask, mask2, old_of, SUMC = preprocess(edge_index)

    scale = 1.0 / math.sqrt(DH)
    wq_s = np.asarray(wq, np.float32) * scale
    wv_p = np.asarray(wv, np.float32).reshape(L, D, H, DH).transpose(0, 1, 3, 2).reshape(L, D, TD)
    wqkv_h = np.concatenate([np.asarray(wk, np.float32), wv_p, wq_s], axis=2)
    wqkv_h = _bf16(wqkv_h.reshape(L, 2, P, 3 * TD))
    w_in_h = _bf16(np.asarray(w_in, np.float32).reshape(2, P, D))
    wo_p = np.asarray(wo, np.float32).reshape(L, H, DH, D).transpose(0, 2, 1, 3).reshape(L, TD, D)
    wo_h = _bf16(wo_p.reshape(L, 2, P, D))
    w1_h = _bf16(np.asarray(w1, np.float32).reshape(L, 2, P, FFN))
    w2_h = _bf16(np.asarray(w2, np.float32).reshape(L, 8, P, D))

    in_maps = []
    for c in range(C):
        xs = _bf16(x[old_of[c], :].T)
        in_maps.append({
            "x_t": np.ascontiguousarray(xs.reshape(2, P, NS)),
            "w_in": w_in_h, "wqkv": wqkv_h, "wo": wo_h, "w1": w1_h, "w2": w2_h,
            "idx16": np.ascontiguousarray(idx16[c]),
            "mask": _bf16(mask[c]), "mask2": _bf16(mask2[c]),
        })
    return meta, SUMC, old_of, in_maps


def assemble_out(results, old_of):
    N, D, H, DH, FFN, L, C, NS, NCH = _dims()
    out = np.empty((N, D), np.float32)
    for c in range(C):
        o = np.asarray(results[c]["out"], np.float32).reshape(2 * P, NS)
        out[old_of[c], :] = o.T
    return out


_BUILD_CACHE = {}


def _get_nc(meta, sumc):
    key = (meta, sumc)
    if key not in _BUILD_CACHE:
        _BUILD_CACHE[key] = build_nc(meta, sumc)
    return _BUILD_CACHE[key]


def kernel(x, edge_index, w_in, b_in, ln1_g, ln1_b, ln2_g, ln2_b,
           wq, bq, wk, bk, wv, bv, wo, bo, w1, b1, w2, b2, _trace=False):
    from concourse.bass_utils import run_bass_kernel_spmd

    for b in (b_in, bq, bk, bv, bo, b1, b2, ln1_b, ln2_b):
        assert np.abs(np.asarray(b)).max() == 0.0, "nonzero bias unsupported"
    for g in (ln1_g, ln2_g):
        assert np.abs(np.asarray(g) - 1.0).max() == 0.0, "non-unit LN gamma unsupported"

    meta, sumc, old_of, in_maps = make_in_maps(x, edge_index, w_in, wq, wk, wv, wo, w1, w2)
    nc = _get_nc(meta, sumc)
    res = run_bass_kernel_spmd(nc, in_maps, core_ids=list(range(CFG["C"])), trace=_trace)
    if _trace:
        kernel._last_result = res
    return assemble_out(res.results, old_of)


# revision 26
# speedup vs baseline: 1.1154x; 1.0002x over previous
"""AllostericGNN Trainium2 kernel (8 NeuronCores, SPMD) — slot-major attention.

Strategy: shard nodes (and their in-edges, grouped by dst) across 8 cores.
Nodes are degree-sorted and striped across cores so all cores see an identical
degree profile (the compiled program is shared); the inverse permutation is
applied on the host at output assembly. Per 128-dst tile, in-edges live in
"slots" along the free axis of the dst partition: the self edge is slot 0 and
edge slots follow, so ONE SWDGE dma_gather per tile-pass covers self+edges
(s-major 16-wrapped int16 indices, single_packet=False, 4 SWDGE queues
round-robin; Q7 descriptor emission is ~5.3ns/idx flat — merging gathers
bought nothing and fp8 wire + upcast-DMA stole SDMA bandwidth from gathers:
both measured dead ends). Pad slots point at valid self rows and are masked
after exp. Scores are a broadcast multiply + log-tree of packed TT adds over
dh (2x mode; the 1x tensor_reduce measured ~2x slower), softmax skips
max-subtraction (|scores| << 1), and V is pre-permuted to dh-major so the
exp-weighting broadcast stays in the DVE 2x packed mode. Per-tile aggregation
is a log-tree of packed adds in a separate product tile (in-place reuse of kvg
measurably regresses). Small copies that would run in DVE 2-port perf mode
(q_all, gp init) go on the Scalar engine: 2-port DVE ops stall 15-20us against
concurrent SWDGE descriptor-ring writes. LayerNorm computes rstd =
Exp(-0.5*Ln(var+eps)) so the Exp activation table is shared with attention.
The residual stream h_T is bf16. The whole schedule is chunk-pipelined (512
nodes): input-proj/LN/QKV per chunk up front, then per chunk attention ->
O-proj/LN2/FFN -> next-layer LN1/QKV -> per-chunk AllGather (AGQ=8) whose
output tile is addr_space=Shared (direct peer writes; the Local-output mesh
path measured 62-78us per 1MB-rank AG).

Measured dead ends (do not retry blindly): multi-offset indirect_dma_start
silently delivers only the first block per partition for non-consecutive rows;
AGQ=2(bf16+Local), ffn bufs=1, att bufs=3, GPAIR=4, SCAP=24-with-inplace,
gather-merging via tile pairing, fp8 AG + DRAM->DRAM upcast-DMA all regressed.
Also regressed: prep+trigger for EVERY gather (+1.07ms — per-pass trigger_dma
and sem-wait overhead dwarfs any emission win); moving the self edge out of
the gather into a local kv_loc DMA + per-tile DVE init (+15us — 9 extra small
DVE/ACT ops per tile cost more than the 5.7%% Q7 saving). prepare_only preps
issued BEFORE an AllGather invert RAW (Tile records the read against the
previous table writer -> NaN); issued after, they are correct but gain nothing
because gpsimd blocks on the collective completion. collective_compute is
gpsimd-only. Run-to-run variance measured ~5-15us on this kernel.
"""
import math
import numpy as np

CFG = dict(N=32768, D=256, H=8, DH=32, FFN=1024, L=2, C=8)
EPS = 1e-5
P = 128
SCAP = 20          # kvg slots per attention pass (self + edges)
KVBUFS = 3
AGQ = 1            # single whole-layer AllGather (Shared output)
NCHUNK = 512
T = 32


def _dims():
    N, D, C = CFG["N"], CFG["D"], CFG["C"]
    NS = N // C
    NCH = NS // NCHUNK
    return N, D, CFG["H"], CFG["DH"], CFG["FFN"], CFG["L"], C, NS, NCH


def _bf16(x):
    import ml_dtypes
    return np.asarray(x).astype(ml_dtypes.bfloat16)


def build_pass_plan(S_t, scap=SCAP):
    """One gather pass per tile; oversized tiles split. Grouped per chunk.

    Returns (plan, gcol): plan[g][p] = [(t, col0, ncols)] and gcol[g][p] =
    global pass-ordered column offset.
    """
    plan = []
    for g in range(T * P // NCHUNK):
        grp = []
        for t in range(g * 4, g * 4 + 4):
            ct = S_t[t] + 1
            k = 0
            while k < ct:
                n1 = ct - k if ct - k <= scap else (ct - k + 1) // 2
                n1 = min(n1, scap)
                grp.append([(t, k, n1)])
                k += n1
        plan.append(grp)
    gcol = []
    run = 0
    for grp in plan:
        offs = []
        for segs in grp:
            offs.append(run)
            run += sum(s[2] for s in segs)
        gcol.append(offs)
    return plan, gcol


def preprocess(edge_index):
    """Degree-sorted striped node permutation + pass-ordered gather tables."""
    N, D, H, DH, FFN_, L_, C, NS, NCH = _dims()
    src0 = np.asarray(edge_index[0], dtype=np.int64)
    dst0 = np.asarray(edge_index[1], dtype=np.int64)
    E = src0.shape[0]
    deg = np.bincount(dst0, minlength=N).astype(np.int64)

    # stripe assignment: rank r (by degree) -> core r%C, slot r//C
    order = np.argsort(deg, kind="stable")
    r = np.arange(N)
    c_of = np.empty(N, np.int64); l0_of = np.empty(N, np.int64)
    c_of[order] = r % C
    l0_of[order] = r // C
    tile_of = l0_of // P
    # within-tile descending degree resort
    keys = np.lexsort((l0_of, -deg, tile_of, c_of))
    pos = np.empty(N, np.int64)
    pos[keys] = np.arange(N)
    p_of = pos % P
    l_of = tile_of * P + p_of

    QCH = NS // AGQ
    trow_of = (l_of // QCH) * (C * QCH) + c_of * QCH + (l_of % QCH)

    # per-edge slot index (occurrence among in-edges of its dst)
    gd = c_of[dst0] * NS + l_of[dst0]
    eorder = np.argsort(gd, kind="stable")
    gds = gd[eorder]
    srcs = src0[eorder]
    bounds = np.searchsorted(gds, np.arange(N + 1))
    slot = np.arange(E) - bounds[gds]

    degmat = np.zeros((C, T, P), np.int64)
    degmat[c_of, tile_of, p_of] = deg
    S_t = tuple(int(s) for s in degmat.max(axis=2).max(axis=0))

    csC = np.concatenate([[0], np.cumsum(np.array(S_t) + 1)])   # tile-major cols
    SUMC = int(csC[-1])

    trow_node = np.zeros((C, T, P), np.int64)
    trow_node[c_of, tile_of, p_of] = trow_of

    # tile-major offsets: col 0 self, pads default to own row
    offs = np.empty((C, P, SUMC), np.int32)
    for t in range(T):
        offs[:, :, csC[t]:csC[t + 1]] = trow_node[:, t, :, None].astype(np.int32)
    dce = c_of[dst0[eorder]]
    dte = tile_of[dst0[eorder]]
    dpe = p_of[dst0[eorder]]
    col = csC[dte] + 1 + slot
    offs[dce, dpe, col] = trow_of[srcs].astype(np.int32)

    mask = np.zeros((C, P, SUMC), np.float32)
    for t in range(T):
        mask[:, :, csC[t]] = 1.0
        s_arr = np.arange(1, S_t[t] + 1)
        mask[:, :, csC[t] + 1:csC[t + 1]] = (degmat[:, t, :, None] >= s_arr).astype(np.float32)
    mask2 = (degmat > 0).astype(np.float32).transpose(0, 2, 1)  # [C, P, T]

    old_of = np.empty((C, NS), np.int64)
    old_of[c_of, l_of] = np.arange(N)

    # pass-ordered column permutation + per-pass 16-wrapped SWDGE idx blocks
    plan, _g = build_pass_plan(S_t)
    idx_parts = []
    perm = []
    for grp in plan:
        for segs in grp:
            cols = []
            for (t, col0, ncols) in segs:
                cols.extend(range(csC[t] + col0, csC[t] + col0 + ncols))
            perm.extend(cols)
            blk = offs[:, :, cols].transpose(0, 2, 1).reshape(C, len(cols) * P)
            w = blk.reshape(C, len(cols) * P // 16, 16).transpose(0, 2, 1)
            idx_parts.append(np.tile(w, (1, 8, 1)))
    idx16 = np.concatenate(idx_parts, axis=2).astype(np.int16)   # [C, 128, SUMC*8]
    mask_p = mask[:, :, perm]
    meta = (S_t,)
    return meta, idx16, mask_p, mask2, old_of, SUMC


def build_nc(meta, sumc, debug=False):
    import concourse.bacc as bacc
    import concourse.mybir as mybir
    import concourse.tile as tile
    from concourse import bass
    from concourse.masks import make_identity

    (S_t,) = meta
    N, D, H, DH, FFN, L, C, NS, NCH = _dims()
    TD = D
    fp32 = mybir.dt.float32
    bf16 = mybir.dt.bfloat16
    AF = mybir.ActivationFunctionType
    OP = mybir.AluOpType
    QCH = NS // AGQ
    plan, gcol = build_pass_plan(S_t)

    nc = bacc.Bacc("TRN2", target_bir_lowering=False, debug=False, num_devices=CFG["C"],
                   num_swdge_queues=4)

    x_t = nc.declare_dram_parameter("x_t", [2, P, NS], bf16, isOutput=False)
    w_in = nc.declare_dram_parameter("w_in", [2, P, D], bf16, isOutput=False)
    wqkv = nc.declare_dram_parameter("wqkv", [L, 2, P, 3 * TD], bf16, isOutput=False)
    wo = nc.declare_dram_parameter("wo", [L, 2, P, D], bf16, isOutput=False)
    w1 = nc.declare_dram_parameter("w1", [L, 2, P, FFN], bf16, isOutput=False)
    w2 = nc.declare_dram_parameter("w2", [L, 8, P, D], bf16, isOutput=False)
    idx_e = nc.declare_dram_parameter("idx16", [P, sumc * 8], mybir.dt.int16, isOutput=False)
    mask_e = nc.declare_dram_parameter("mask", [P, sumc], bf16, isOutput=False)
    mask2_e = nc.declare_dram_parameter("mask2", [P, T], bf16, isOutput=False)
    out_e = nc.declare_dram_parameter("out", [2, P, NS], bf16, isOutput=True)

    with nc.allow_low_precision("bf16 residual/score"), tile.TileContext(nc) as tc:
        with (
            tc.tile_pool(name="persist", bufs=1) as pp,
            tc.tile_pool(name="dram", bufs=1, space="DRAM") as dp,
            tc.tile_pool(name="kvgp", bufs=KVBUFS) as kp,
            tc.tile_pool(name="prodp", bufs=1) as rp,
            tc.tile_pool(name="att", bufs=2) as ap,
            tc.tile_pool(name="ln", bufs=2) as lp,
            tc.tile_pool(name="xin", bufs=2) as xp,
            tc.tile_pool(name="ffn", bufs=2) as fp,
            tc.tile_pool(name="ps_tr", bufs=2, space="PSUM") as ps_tr,
            tc.tile_pool(name="ps_ln", bufs=2, space="PSUM") as ps_ln,
            tc.tile_pool(name="ps_g", bufs=2, space="PSUM") as ps_g,
        ):
            from concourse import library_config
            nc.gpsimd.load_library(library_config.mlp)

            # ---- persistent SBUF ----
            h_T = pp.tile([P, 2, NS], bf16)
            act_T = pp.tile([P, 2, NS], bf16)
            w_in_sb = pp.tile([P, 2, D], bf16)
            wqkv_sb = pp.tile([P, L, 2, 3 * TD], bf16)
            wo_sb = pp.tile([P, L, 2, D], bf16)
            w1_sb = pp.tile([P, L, 2, FFN], bf16)
            w2_sb = pp.tile([P, L, 8, D], bf16)
            idx_sb = pp.tile([P, sumc * 8], mybir.dt.int16)
            mask_sb = pp.tile([P, sumc], bf16)
            mask2_sb = pp.tile([P, T], bf16)
            q_all = pp.tile([P, T, TD], bf16)
            ones_b = pp.tile([P, P], bf16)
            epsb = pp.tile([P, 1], fp32)
            ident = pp.tile([P, P], bf16)

            nc.sync.dma_start(w_in_sb[:], w_in[:].rearrange("c p d -> p c d"))
            nc.sync.dma_start(wqkv_sb[:], wqkv[:].rearrange("l c p d -> p l c d"))
            nc.sync.dma_start(wo_sb[:], wo[:].rearrange("l c p d -> p l c d"))
            nc.sync.dma_start(w1_sb[:], w1[:].rearrange("l c p d -> p l c d"))
            nc.sync.dma_start(w2_sb[:], w2[:].rearrange("l c p d -> p l c d"))
            nc.sync.dma_start(idx_sb[:], idx_e[:])
            nc.sync.dma_start(mask_sb[:], mask_e[:])
            nc.sync.dma_start(mask2_sb[:], mask2_e[:])
            nc.vector.memset(ones_b[:], 1.0 / D)
            nc.vector.memset(epsb[:], EPS)
            make_identity(nc, ident[:])

            kv_loc = [[dp.tile([QCH, 2 * TD], bf16, tag=f"kvloc{l}_{q}", name=f"kvloc{l}_{q}")
                       for q in range(AGQ)] for l in range(L)]
            kv_tab = [dp.tile([N, 2 * TD], bf16, tag=f"kvtab{l}", name=f"kvtab{l}",
                              addr_space="Shared")
                      for l in range(L)]

            # pre-touch kvg slots so pad partitions hold finite stale data
            for _i in range(KVBUFS):
                kvg_init = kp.tile([P, SCAP * 2 * TD], bf16, tag="kvg", name=f"kvg_init{_i}")
                nc.vector.memset(kvg_init[:], 0.0)

            def layernorm_chunk(src, dst, nchk):
                ns = slice(nchk * NCHUNK, (nchk + 1) * NCHUNK)
                mu_p = ps_ln.tile([P, NCHUNK], fp32, space="PSUM", tag="mu", name="mu_p")
                ex2_p = ps_ln.tile([P, NCHUNK], fp32, space="PSUM", tag="ex2", name="ex2_p")
                sq = lp.tile([P, 2, NCHUNK], bf16, tag="sq", name="sq")
                for c in range(2):
                    nc.scalar.activation(sq[:, c, :], src[:, c, ns], AF.Square)
                for c in range(2):
                    nc.tensor.matmul(mu_p[:], lhsT=ones_b[:], rhs=src[:, c, ns],
                                     start=(c == 0), stop=(c == 1))
                    nc.tensor.matmul(ex2_p[:], lhsT=ones_b[:], rhs=sq[:, c, :],
                                     start=(c == 0), stop=(c == 1))
                mu_sb = lp.tile([P, NCHUNK], bf16, tag="musb", name="mu_sb")
                nc.scalar.copy(mu_sb[:], mu_p[:])
                mu2b = lp.tile([P, NCHUNK], bf16, tag="mu2b", name="mu2b")
                nc.scalar.activation(mu2b[:], mu_p[:], AF.Square)
                mu2 = lp.tile([P, NCHUNK], bf16, tag="mu2", name="mu2")
                nc.vector.tensor_tensor(out=mu2[:], in0=ex2_p[:], in1=mu2b[:], op=OP.subtract)
                lg = lp.tile([P, NCHUNK], bf16, tag="sd", name="lg")
                nc.scalar.activation(lg[:], mu2[:], AF.Ln, bias=epsb[:])
                rstd = lp.tile([P, NCHUNK], bf16, tag="rstd", name="rstd")
                nc.scalar.activation(rstd[:], lg[:], AF.Exp, scale=-0.5)
                ms = lp.tile([P, NCHUNK], bf16, tag="ms", name="ms")
                nc.vector.tensor_tensor(out=ms[:], in0=mu_sb[:], in1=rstd[:], op=OP.mult)
                for c in range(2):
                    tmp = lp.tile([P, NCHUNK], bf16, tag="tmp", name="tmp")
                    nc.vector.tensor_tensor(out=tmp[:], in0=src[:, c, ns], in1=rstd[:], op=OP.mult)
                    nc.vector.tensor_tensor(out=dst[:, c, ns], in0=tmp[:], in1=ms[:], op=OP.subtract)

            def inputproj_chunk(nchk):
                ns = slice(nchk * NCHUNK, (nchk + 1) * NCHUNK)
                xin = xp.tile([P, 2, NCHUNK], bf16, tag="xin", name="xin")
                nc.sync.dma_start(xin[:], x_t[:, :, ns].rearrange("c p n -> p c n"))
                for co in range(2):
                    hp = ps_g.tile([P, NCHUNK], fp32, space="PSUM", tag="gemm", name="hp")
                    for ck in range(2):
                        nc.tensor.matmul(hp[:], lhsT=w_in_sb[:, ck, co * P:(co + 1) * P],
                                         rhs=xin[:, ck, :], start=(ck == 0), stop=(ck == 1))
                    nc.scalar.copy(h_T[:, co, ns], hp[:])

            def kv_tile(l, t):
                tsl = slice(t * P, (t + 1) * P)
                kv_b = ap.tile([P, 2 * TD], bf16, tag="kvb", name="kv_b")
                kv_p = ps_g.tile([P, 512], fp32, space="PSUM", tag="gemm", name="kv_p")
                for ck in range(2):
                    nc.tensor.matmul(kv_p[:], lhsT=act_T[:, ck, tsl],
                                     rhs=wqkv_sb[:, l, ck, 0:512],
                                     start=(ck == 0), stop=(ck == 1))
                nc.scalar.copy(kv_b[:], kv_p[:])
                q = t // (T // AGQ)
                r0 = (t % (T // AGQ)) * P
                nc.sync.dma_start(kv_loc[l][q][r0:r0 + P, :], kv_b[:])

            def q_tile(l, t):
                tsl = slice(t * P, (t + 1) * P)
                q_p = ps_g.tile([P, 512], fp32, space="PSUM", tag="gemm", name="q_p")
                for ck in range(2):
                    nc.tensor.matmul(q_p[:, 0:TD], lhsT=act_T[:, ck, tsl],
                                     rhs=wqkv_sb[:, l, ck, 512:768],
                                     start=(ck == 0), stop=(ck == 1))
                nc.scalar.copy(q_all[:, t, :], q_p[:, 0:TD])

            def allgather_chunk(l, q):
                rows = slice(q * C * QCH, (q + 1) * C * QCH)
                nc.gpsimd.collective_compute(
                    "AllGather", mybir.AluOpType.bypass,
                    ins=[kv_loc[l][q][:].rearrange("(u r) d -> u (r d)", u=1)],
                    outs=[kv_tab[l][rows, :].rearrange("(u r) d -> u (r d)", u=1)],
                    replica_groups=[list(range(C))],
                )

            qrr = [0]
            tstate = {}

            def issue_gather(l, segs, gc, prep_sem=None):
                nct = sum(s[2] for s in segs)
                kvg = kp.tile([P, SCAP * 2 * TD], bf16, tag="kvg", name="kvg")
                kv3 = kvg[:].rearrange("p (s d) -> p s d", d=2 * TD)
                qn = qrr[0] % 4
                nc.gpsimd.dma_gather(
                    kv3[:, 0:nct, :],
                    kv_tab[l][:],
                    idx_sb[:, gc * 8:(gc + nct) * 8],
                    nct * P, nct * P, 2 * TD,
                    single_packet=False,
                    queue_num=qn,
                    prepare_only=(prep_sem is not None),
                    sem=prep_sem,
                )
                qrr[0] += 1
                return kvg, qn

            NPREF = KVBUFS
            pref = []
            psem = [0]

            def prefetch_layer(l):
                flat = [(segs, gcol[g][pi]) for g in range(NCH)
                        for pi, segs in enumerate(plan[g])]
                for i in range(NPREF):
                    sem = nc.alloc_semaphore(f"pref{psem[0]}")
                    psem[0] += 1
                    kvg, qn = issue_gather(l, flat[i][0], flat[i][1], prep_sem=sem)
                    pref.append((kvg, qn, sem))

            def fire_prefetch():
                for _kvg, qn, _sem in pref:
                    nc.gpsimd.trigger_dma(count=None, queue_num=qn)

            def attention_pass(l, segs, gc, kvg=None, dsem=None):
                if kvg is None:
                    kvg, _ = issue_gather(l, segs, gc)
                if dsem is not None:
                    nc.vector.wait_ge(dsem, 16)
                kv3 = kvg[:].rearrange("p (s d) -> p s d", d=2 * TD)
                prod = rp.tile([P, SCAP, TD], bf16, tag="prod", name="prod")
                sc = ap.tile([P, SCAP * H], bf16, tag="sc", name="sc")
                exm = ap.tile([P, SCAP * H], bf16, tag="exm", name="exm")
                off = 0
                for (t, col0, ncols) in segs:
                    k3 = kv3[:, off:off + ncols, 0:TD]
                    v3 = kv3[:, off:off + ncols, TD:2 * TD]
                    CT = S_t[t] + 1
                    first, last = col0 == 0, col0 + ncols == CT
                    if first:
                        z_t = ap.tile([P, H], fp32, tag=f"z{t % 2}", name="z_t")
                        agg_t = ap.tile([P, TD], fp32, tag=f"agg{t % 2}", name="agg_t")
                        tstate[t] = (z_t, agg_t)
                    else:
                        z_t, agg_t = tstate[t]
                    # scores: prod = K * q_bcast ; tree-reduce over dh (2x TT adds)
                    pr = prod[:, off:off + ncols, :]
                    nc.vector.tensor_tensor(
                        out=pr[:], in0=k3[:],
                        in1=q_all[:, t, :].rearrange("p (s d) -> p s d", s=1).to_broadcast([P, ncols, TD]),
                        op=OP.mult)
                    pv = pr[:].rearrange("p s (h d) -> p (s h) d", h=H)
                    m = DH
                    while m > 2:
                        a = m // 2
                        nc.vector.tensor_tensor(
                            out=pv[:, :, 0:a], in0=pv[:, :, 0:a], in1=pv[:, :, a:m],
                            op=OP.add)
                        m = a
                    scs = sc[:, off * H:(off + ncols) * H]
                    exs = exm[:, off * H:(off + ncols) * H]
                    nc.vector.tensor_tensor(
                        out=scs[:], in0=pv[:, :, 0], in1=pv[:, :, 1], op=OP.add)
                    nc.scalar.activation(exs[:], scs[:], AF.Exp)
                    # mask pads (stale rows beyond per-core validity)
                    nc.vector.tensor_tensor(
                        out=exs[:].rearrange("p (s h) -> p s h", h=H),
                        in0=exs[:].rearrange("p (s h) -> p s h", h=H),
                        in1=mask_sb[:, gc + off:gc + off + ncols]
                            .rearrange("p (s u) -> p s u", u=1).to_broadcast([P, ncols, H]),
                        op=OP.mult)
                    # z partial
                    zp = z_t if first else ap.tile([P, H], fp32, tag="zp", name="zp")
                    nc.vector.tensor_reduce(
                        out=zp[:],
                        in_=exs[:].rearrange("p (s h) -> p h s", h=H),
                        axis=mybir.AxisListType.X, op=OP.add)
                    if not first:
                        nc.vector.tensor_tensor(out=z_t[:], in0=z_t[:], in1=zp[:], op=OP.add)
                    # weighted V: prod = V * exp_bcast (V is dh-major so h packs last)
                    nc.vector.tensor_tensor(
                        out=pr[:].rearrange("p s (u h) -> p s u h", h=H),
                        in0=v3[:].rearrange("p s (u h) -> p s u h", h=H),
                        in1=exs[:].rearrange("p (s u h) -> p s u h", h=H, u=1)
                            .to_broadcast([P, ncols, DH, H]),
                        op=OP.mult)
                    # tree-sum over slots -> agg partial
                    m = ncols
                    while m > 2:
                        a = m - m // 2
                        nc.vector.tensor_tensor(
                            out=pr[:, 0:m - a, :], in0=pr[:, 0:m - a, :],
                            in1=pr[:, a:m, :], op=OP.add)
                        m = a
                    gp = agg_t if first else ap.tile([P, TD], fp32, tag="gp", name="gp")
                    if m == 2:
                        nc.vector.tensor_tensor(out=gp[:], in0=pr[:, 0, :], in1=pr[:, 1, :], op=OP.add)
                    else:
                        nc.scalar.copy(gp[:], pr[:, 0, :])
                    if not first:
                        nc.vector.tensor_tensor(out=agg_t[:], in0=agg_t[:], in1=gp[:], op=OP.add)
                    if last:
                        finalize_tile(t, z_t, agg_t)
                    off += ncols

            def finalize_tile(t, z_t, agg_t):
                tsl = slice(t * P, (t + 1) * P)
                rz = ap.tile([P, H], fp32, tag="rz", name="rz")
                nc.vector.reciprocal(rz[:], z_t[:])
                rzm = ap.tile([P, H], bf16, tag="rzm", name="rzm")
                nc.vector.tensor_tensor(out=rzm[:], in0=rz[:],
                                        in1=mask2_sb[:, t:t + 1].to_broadcast([P, H]), op=OP.mult)
                att = ap.tile([P, TD], bf16, tag="att", name="att")
                nc.vector.tensor_tensor(
                    out=att[:].rearrange("p (u h) -> p u h", h=H),
                    in0=agg_t[:].rearrange("p (u h) -> p u h", h=H),
                    in1=rzm[:].rearrange("p (u h) -> p u h", u=1).to_broadcast([P, DH, H]),
                    op=OP.mult)
                trp = ps_tr.tile([P, 2, P], bf16, space="PSUM", tag="tr", name="trp")
                for c in range(2):
                    nc.tensor.transpose(trp[:, c, :], att[:, c * P:(c + 1) * P], ident[:])
                    nc.scalar.copy(act_T[:, c, tsl], trp[:, c, :])

            def oproj_chunk(l, nchk):
                ns = slice(nchk * NCHUNK, (nchk + 1) * NCHUNK)
                for co in range(2):
                    op_p = ps_g.tile([P, NCHUNK], fp32, space="PSUM", tag="gemm", name="op_p")
                    for ck in range(2):
                        nc.tensor.matmul(op_p[:], lhsT=wo_sb[:, l, ck, co * P:(co + 1) * P],
                                         rhs=act_T[:, ck, ns], start=(ck == 0), stop=(ck == 1))
                    op_b = lp.tile([P, NCHUNK], bf16, tag="resid", name="op_b")
                    nc.scalar.copy(op_b[:], op_p[:])
                    nc.vector.tensor_tensor(out=h_T[:, co, ns], in0=h_T[:, co, ns], in1=op_b[:], op=OP.add)

            def ffn_chunk(l, nchk):
                ns = slice(nchk * NCHUNK, (nchk + 1) * NCHUNK)
                h1 = fp.tile([P, 8, NCHUNK], bf16, tag="h1", name="h1")
                for m in range(8):
                    g1 = ps_g.tile([P, NCHUNK], fp32, space="PSUM", tag="gemm", name="g1")
                    for ck in range(2):
                        nc.tensor.matmul(g1[:], lhsT=w1_sb[:, l, ck, m * P:(m + 1) * P],
                                         rhs=act_T[:, ck, ns], start=(ck == 0), stop=(ck == 1))
                    nc.scalar.activation(h1[:, m, :], g1[:], AF.Gelu)
                for co in range(2):
                    g2 = ps_g.tile([P, NCHUNK], fp32, space="PSUM", tag="gemm", name="g2")
                    for ck in range(8):
                        nc.tensor.matmul(g2[:], lhsT=w2_sb[:, l, ck, co * P:(co + 1) * P],
                                         rhs=h1[:, ck, :], start=(ck == 0), stop=(ck == 7))
                    g2_b = lp.tile([P, NCHUNK], bf16, tag="resid", name="g2_b")
                    nc.scalar.copy(g2_b[:], g2[:])
                    nc.vector.tensor_tensor(out=h_T[:, co, ns], in0=h_T[:, co, ns], in1=g2_b[:], op=OP.add)

            TPC = NCHUNK // P  # tiles per node-chunk (4)

            # ---- layer 0 front: input-proj sweep, LN1+KV per chunk, AG,
            # then all Q GEMMs overlapping the AllGather ----
            for g in range(NCH):
                inputproj_chunk(g)
            for g in range(NCH):
                layernorm_chunk(h_T, act_T, g)
                for t in range(g * TPC, (g + 1) * TPC):
                    kv_tile(0, t)
            allgather_chunk(0, 0)
            prefetch_layer(0)
            fire_prefetch()
            for t in range(T):
                q_tile(0, t)

            for l in range(L):
                pcnt = [0]
                for g in range(NCH):
                    for pi, segs in enumerate(plan[g]):
                        if pcnt[0] < NPREF:
                            attention_pass(l, segs, gcol[g][pi], kvg=pref[0][0],
                                           dsem=pref[0][2])
                            pref.pop(0)
                        else:
                            attention_pass(l, segs, gcol[g][pi])
                        pcnt[0] += 1
                    oproj_chunk(l, g)
                    layernorm_chunk(h_T, act_T, g)
                    ffn_chunk(l, g)
                    if l + 1 < L:
                        layernorm_chunk(h_T, act_T, g)   # LN1 of next layer
                        for t in range(g * TPC, (g + 1) * TPC):
                            kv_tile(l + 1, t)
                        if g == NCH - 1:
                            allgather_chunk(l + 1, 0)
                            prefetch_layer(l + 1)
                            fire_prefetch()
                            for t2 in range(T):
                                q_tile(l + 1, t2)
                    else:
                        ns = slice(g * NCHUNK, (g + 1) * NCHUNK)
                        for c in range(2):
                            nc.sync.dma_start(out_e[c, :, ns], h_T[:, c, ns])

    nc.compile()
    return nc


def make_in_maps(x, edge_index, w_in, wq, wk, wv, wo, w1, w2):
    N, D, H, DH, FFN, L, C, NS, NCH = _dims()
    TD = D
    x = np.asarray(x, np.float32)
    meta, idx16, mask, mask2, old_of, SUMC = preprocess(edge_index)

    scale = 1.0 / math.sqrt(DH)
    wq_s = np.asarray(wq, np.float32) * scale
    wv_p = np.asarray(wv, np.float32).reshape(L, D, H, DH).transpose(0, 1, 3, 2).reshape(L, D, TD)
    wqkv_h = np.concatenate([np.asarray(wk, np.float32), wv_p, wq_s], axis=2)
    wqkv_h = _bf16(wqkv_h.reshape(L, 2, P, 3 * TD))
    w_in_h = _bf16(np.asarray(w_in, np.float32).reshape(2, P, D))
    wo_p = np.asarray(wo, np.float32).reshape(L, H, DH, D).transpose(0, 2, 1, 3).reshape(L, TD, D)
    wo_h = _bf16(wo_p.reshape(L, 2, P, D))
    w1_h = _bf16(np.asarray(w1, np.float32).reshape(L, 2, P, FFN))
    w2_h = _bf16(np.asarray(w2, np.float32).reshape(L, 8, P, D))

    in_maps = []
    for c in range(C):
        xs = _bf16(x[old_of[c], :].T)
        in_maps.append({
            "x_t": np.ascontiguousarray(xs.reshape(2, P, NS)),
            "w_in": w_in_h, "wqkv": wqkv_h, "wo": wo_h, "w1": w1_h, "w2": w2_h,
            "idx16": np.ascontiguousarray(idx16[c]),
            "mask": _bf16(mask[c]), "mask2": _bf16(mask2[c]),
        })
    return meta, SUMC, old_of, in_maps


def assemble_out(results, old_of):
    N, D, H, DH, FFN, L, C, NS, NCH = _dims()
    out = np.empty((N, D), np.float32)
    for c in range(C):
        o = np.asarray(results[c]["out"], np.float32).reshape(2 * P, NS)
        out[old_of[c], :] = o.T
    return out


_BUILD_CACHE = {}


def _get_nc(meta, sumc):
    key = (meta, sumc)
    if key not in _BUILD_CACHE:
        _BUILD_CACHE[key] = build_nc(meta, sumc)
    return _BUILD_CACHE[key]


def kernel(x, edge_index, w_in, b_in, ln1_g, ln1_b, ln2_g, ln2_b,
           wq, bq, wk, bk, wv, bv, wo, bo, w1, b1, w2, b2, _trace=False):
    from concourse.bass_utils import run_bass_kernel_spmd

    for b in (b_in, bq, bk, bv, bo, b1, b2, ln1_b, ln2_b):
        assert np.abs(np.asarray(b)).max() == 0.0, "nonzero bias unsupported"
    for g in (ln1_g, ln2_g):
        assert np.abs(np.asarray(g) - 1.0).max() == 0.0, "non-unit LN gamma unsupported"

    meta, sumc, old_of, in_maps = make_in_maps(x, edge_index, w_in, wq, wk, wv, wo, w1, w2)
    nc = _get_nc(meta, sumc)
    res = run_bass_kernel_spmd(nc, in_maps, core_ids=list(range(CFG["C"])), trace=_trace)
    if _trace:
        kernel._last_result = res
    return assemble_out(res.results, old_of)
